# revision 1
# baseline (speedup 1.0000x reference)
"""Trainium2 Bass kernel for nn_DelayedMLP (B=32, S=2048, I=256, H=512, O=256).

Strategy
--------
Sequence-parallel decomposition of the recurrent scan: the buffer state's
dependence on the past decays geometrically (|d buf_t / d buf_{t-w}| ~ 0.5^w),
so a chain started from buf=0 a few steps early converges to the true state to
fp32 precision.  Each of the 8 cores takes a 256-step S-chunk; within a core
the chunk is split into 16 chains of 16 steps, each warmed up for 16 steps.
All 16 chains advance in lockstep, vectorized with the batch (16*32 = 512
tokens per device step), which makes the per-step gate matmul a full-width
[128,128] x [128,256] operation instead of a [4,256] sliver.

Algebraic simplifications used:
  e_t   = x_t * sigmoid(-(x_t@Wg + bg))          (input gate, bulk-precomputable)
  u_t   = buf_{t-1} + e_t
  buf_t = u_t * sigmoid(-(u_t@Wg + bg))
  out_t = x_t + buf_{t-1} - buf_t                 (imm + release telescopes)

The pointwise MLP runs fused in the same pass over each step's 512 tokens.
All matmul operands are fp16 (same 10-bit mantissa as TF32/fp32r, but 16-bit
dtypes get fast background weight loads and 2x DVE throughput); accumulation
is fp32 in PSUM.  Host-side work is layout only: gather/pad/transpose shards,
un-transpose the result.
"""

import numpy as np
from contextlib import ExitStack

import concourse.bass as bass
import concourse.bacc as bacc
import concourse.tile as tile
from concourse import mybir
from concourse.bass_utils import run_bass_kernel_spmd

F32 = mybir.dt.float32
F32R = mybir.dt.float32r
F16 = mybir.dt.float16

B, S, I, H, O = 32, 2048, 256, 512, 256
NCORES = 8
CHUNK = S // NCORES          # 256 timesteps per core
NCHAIN = 16                  # chains per core
CLEN = CHUNK // NCHAIN       # 16 chunk steps per chain
WARM = 16                    # warmup steps per chain
LSTEP = WARM + CLEN          # 32 device steps
TOK = NCHAIN * B             # 512 tokens per device step
FREE = 2 * TOK               # 1024 = two I-chunk segments


def build_kernel():
    nc = bacc.Bacc("TRN2", target_bir_lowering=False, debug=False)

    xT = nc.dram_tensor("xT", [128, LSTEP, FREE], F16, kind="ExternalInput").ap()
    wg_d = nc.dram_tensor("Wg", [I, I], F16, kind="ExternalInput").ap()
    w1_d = nc.dram_tensor("W1", [I, H], F16, kind="ExternalInput").ap()
    w2_d = nc.dram_tensor("W2", [H, H], F16, kind="ExternalInput").ap()
    w3_d = nc.dram_tensor("W3", [H, O], F16, kind="ExternalInput").ap()
    nbg_d = nc.dram_tensor("nbg", [2, 128, 1], F32, kind="ExternalInput").ap()
    b1_d = nc.dram_tensor("b1c", [4, 128, 1], F32, kind="ExternalInput").ap()
    b2_d = nc.dram_tensor("b2c", [4, 128, 1], F32, kind="ExternalInput").ap()
    b3_d = nc.dram_tensor("b3c", [2, 128, 1], F32, kind="ExternalInput").ap()
    outT = nc.dram_tensor("outT", [128, CLEN, FREE], F32, kind="ExternalOutput").ap()

    SIG = mybir.ActivationFunctionType.Sigmoid
    RELU = mybir.ActivationFunctionType.Relu
    ADD = mybir.AluOpType.add
    MAX = mybir.AluOpType.max

    with tile.TileContext(nc) as tc, ExitStack() as ctx:
        wpool = ctx.enter_context(tc.tile_pool(name="weights", bufs=1))
        xpool = ctx.enter_context(tc.tile_pool(name="xt", bufs=5))
        sdpool = ctx.enter_context(tc.tile_pool(name="sd", bufs=4))
        epool = ctx.enter_context(tc.tile_pool(name="e", bufs=5))
        upool = ctx.enter_context(tc.tile_pool(name="u", bufs=3))
        spool = ctx.enter_context(tc.tile_pool(name="s", bufs=3))
        bpool = ctx.enter_context(tc.tile_pool(name="buf", bufs=4))
        cpool = ctx.enter_context(tc.tile_pool(name="c", bufs=2))
        ctpool = ctx.enter_context(tc.tile_pool(name="ctmp", bufs=2))
        h1pool = ctx.enter_context(tc.tile_pool(name="h1", bufs=4))
        h2pool = ctx.enter_context(tc.tile_pool(name="h2", bufs=3))
        opool = ctx.enter_context(tc.tile_pool(name="osb", bufs=2))
        pd = ctx.enter_context(tc.tile_pool(name="pd", bufs=3, space="PSUM"))
        pz = ctx.enter_context(tc.tile_pool(name="pz", bufs=2, space="PSUM"))
        pm = ctx.enter_context(tc.tile_pool(name="pm", bufs=3, space="PSUM"))

        # --- resident weights: one DMA per matrix, sliced into lhsT blocks ---
        def load_blocks(src, kk, cols, name):
            t = wpool.tile([128, kk * cols], F16, tag=name, name=name)
            nc.sync.dma_start(
                t[:].rearrange("p (k c) -> p k c", k=kk),
                src.rearrange("(k p) c -> p k c", p=128),
            )
            return {
                (k, m): t[:, k * cols + m * 128:k * cols + (m + 1) * 128]
                for k in range(kk)
                for m in range(cols // 128)
            }

        wg = load_blocks(wg_d, 2, I, "wgt")

        def load_bias(src, n, name):
            t = wpool.tile([128, n], F32, tag=name, name=name)
            nc.sync.dma_start(
                t[:].rearrange("p (m one) -> p m one", one=1),
                src.rearrange("m p one -> p m one"),
            )
            return [t[:, m:m + 1] for m in range(n)]

        nbg = load_bias(nbg_d, 2, "nbgt")

        def emit_gate(t):
            """Load x^T(t) and compute the input gate + e(t).  Warmup-phase
            gate psums borrow the MLP pool (idle until the chunk phase)."""
            xt = xpool.tile([128, FREE], F16, tag="xt", name=f"xt{t}")
            nc.sync.dma_start(xt[:], xT[:, t, :])
            gp = (pm if t % 2 else pd) if t < WARM else pd
            zd = [gp.tile([128, TOK], F32, tag=gp is pm and "pm" or "pd", name=f"zd{t}_{i}") for i in range(2)]
            for m in range(2):
                for k in range(2):
                    nc.tensor.matmul(
                        zd[m][:],
                        wg[(k, m)],
                        xt[:, k * TOK:(k + 1) * TOK],
                        start=(k == 0),
                        stop=(k == 1),
                    )
            sd = sdpool.tile([128, FREE], F16, tag="sd", name=f"sd{t}")
            for m in range(2):
                nc.scalar.activation(
                    sd[:, m * TOK:(m + 1) * TOK], zd[m][:], SIG,
                    bias=nbg[m], scale=-1.0,
                )
            e = epool.tile([128, FREE], F16, tag="e", name=f"e{t}")
            for h in range(2):
                sl = slice(h * TOK, (h + 1) * TOK)
                nc.vector.tensor_mul(e[:, sl], xt[:, sl], sd[:, sl])
            return xt, e

        buf_prev = None
        gates = [emit_gate(0), emit_gate(1)]
        w1 = load_blocks(w1_d, 2, H, "w1t")
        w2 = load_blocks(w2_d, 4, H, "w2t")
        w3 = load_blocks(w3_d, 4, O, "w3t")
        b1c = load_bias(b1_d, 4, "b1t")
        b2c = load_bias(b2_d, 4, "b2t")
        b3c = load_bias(b3_d, 2, "b3t")
        for t in range(LSTEP):
            xt, e = gates[t]
            if t + 2 < LSTEP:
                gates.append(emit_gate(t + 2))

            # --- state update ------------------------------------------------
            u = upool.tile([128, FREE], F16, tag="u", name=f"u{t}")
            for h in range(2):
                sl = slice(h * TOK, (h + 1) * TOK)
                if t == 0:
                    nc.vector.tensor_scalar_add(u[:, sl], e[:, sl], 0.0)
                else:
                    nc.vector.tensor_add(u[:, sl], buf_prev[:, sl], e[:, sl])

            zz = [pz.tile([128, TOK], F32, tag="pz", name=f"zz{t}_{i}") for i in range(2)]
            for m in range(2):
                for k in range(2):
                    nc.tensor.matmul(
                        zz[m][:],
                        wg[(k, m)],
                        u[:, k * TOK:(k + 1) * TOK],
                        start=(k == 0),
                        stop=(k == 1),
                    )
            s = spool.tile([128, FREE], F16, tag="s", name=f"s{t}")
            for m in range(2):
                nc.scalar.activation(
                    s[:, m * TOK:(m + 1) * TOK], zz[m][:], SIG,
                    bias=nbg[m], scale=-1.0,
                )
            buf = bpool.tile([128, FREE], F16, tag="buf", name=f"buf{t}")
            for h in range(2):
                sl = slice(h * TOK, (h + 1) * TOK)
                nc.vector.tensor_mul(buf[:, sl], u[:, sl], s[:, sl])

            if t >= WARM:
                # --- combined output: c = x + buf_prev - buf -----------------
                ct = ctpool.tile([128, FREE], F16, tag="ctmp", name=f"ct{t}")
                c = cpool.tile([128, FREE], F16, tag="c", name=f"c{t}")
                for h in range(2):
                    sl = slice(h * TOK, (h + 1) * TOK)
                    nc.gpsimd.tensor_sub(ct[:, sl], buf_prev[:, sl], buf[:, sl])
                    nc.vector.tensor_add(c[:, sl], ct[:, sl], xt[:, sl])

                # --- MLP layer 1: h1 = relu(c @ W1 + b1) ---------------------
                h1 = h1pool.tile([128, 4 * TOK], F16, tag="h1", name=f"h1_{t}")
                for m in range(4):
                    ph = pm.tile([128, TOK], F32, tag="pm", name=f"p1_{t}_{m}")
                    for k in range(2):
                        nc.tensor.matmul(
                            ph[:],
                            w1[(k, m)],
                            c[:, k * TOK:(k + 1) * TOK],
                            start=(k == 0),
                            stop=(k == 1),
                        )
                    nc.scalar.activation(
                        h1[:, m * TOK:(m + 1) * TOK], ph[:], RELU, bias=b1c[m]
                    )

                # --- MLP layer 2: h2 = relu(h1 @ W2 + b2) --------------------
                h2 = h2pool.tile([128, 4 * TOK], F16, tag="h2", name=f"h2_{t}")
                for m in range(4):
                    ph = pm.tile([128, TOK], F32, tag="pm", name=f"p2_{t}_{m}")
                    for k in range(4):
                        nc.tensor.matmul(
                            ph[:],
                            w2[(k, m)],
                            h1[:, k * TOK:(k + 1) * TOK],
                            start=(k == 0),
                            stop=(k == 3),
                        )
                    nc.vector.tensor_scalar(
                        h2[:, m * TOK:(m + 1) * TOK], ph[:],
                        b2c[m], 0.0, op0=ADD, op1=MAX,
                    )

                # --- MLP layer 3: o = h2 @ W3 + b3 ---------------------------
                osb = opool.tile([128, FREE], F32, tag="osb", name=f"osb{t}")
                for m in range(2):
                    ph = pm.tile([128, TOK], F32, tag="pm", name=f"p3_{t}_{m}")
                    for k in range(4):
                        nc.tensor.matmul(
                            ph[:],
                            w3[(k, m)],
                            h2[:, k * TOK:(k + 1) * TOK],
                            start=(k == 0),
                            stop=(k == 3),
                        )
                    nc.vector.tensor_scalar_add(
                        osb[:, m * TOK:(m + 1) * TOK], ph[:], b3c[m]
                    )
                    nc.sync.dma_start(
                        outT[:, t - WARM, m * TOK:(m + 1) * TOK],
                        osb[:, m * TOK:(m + 1) * TOK],
                    )

            buf_prev = buf

    nc.compile()
    return nc


def shard_inputs(x, Wg, bg, W1, b1, W2, b2, W3, b3):
    """Pure layout work: build the per-core transposed/gathered input dict."""
    x = np.ascontiguousarray(np.asarray(x, np.float16))
    xp = np.pad(x, ((0, 0), (WARM, 0), (0, 0)))  # [B, WARM+S, I]

    common = {
        "Wg": np.ascontiguousarray(np.asarray(Wg, np.float16)),
        "W1": np.ascontiguousarray(np.asarray(W1, np.float16)),
        "W2": np.ascontiguousarray(np.asarray(W2, np.float16)),
        "W3": np.ascontiguousarray(np.asarray(W3, np.float16)),
        "nbg": np.ascontiguousarray((-np.asarray(bg, np.float32)).reshape(2, 128, 1)),
        "b1c": np.ascontiguousarray(np.asarray(b1, np.float32).reshape(4, 128, 1)),
        "b2c": np.ascontiguousarray(np.asarray(b2, np.float32).reshape(4, 128, 1)),
        "b3c": np.ascontiguousarray(np.asarray(b3, np.float32).reshape(2, 128, 1)),
    }

    in_maps = []
    for k in range(NCORES):
        # window[b, j, t, i] = xp[b, k*CHUNK + j*CLEN + t, i]
        starts = k * CHUNK + np.arange(NCHAIN) * CLEN
        idx = starts[:, None] + np.arange(LSTEP)[None, :]  # [j, t]
        win = xp[:, idx, :]                                # [B, j, t, I]
        win = win.reshape(B, NCHAIN, LSTEP, 2, 128)        # [b, j, t, seg, p]
        xTc = win.transpose(4, 2, 3, 1, 0).reshape(128, LSTEP, FREE)
        in_maps.append({"xT": np.ascontiguousarray(xTc), **common})
    return in_maps


def unshard_output(results):
    out = np.empty((B, S, O), np.float32)
    for k in range(NCORES):
        r_ = results[k]["outT"].reshape(128, CLEN, 2, NCHAIN, B)
        # [p, tc, seg, j, b] -> [b, j, tc, seg, p]
        blk = r_.transpose(4, 3, 1, 2, 0).reshape(B, CHUNK, O)
        out[:, k * CHUNK:(k + 1) * CHUNK, :] = blk
    return out


_NC_CACHE = {}


def _get_nc():
    if "nc" not in _NC_CACHE:
        _NC_CACHE["nc"] = build_kernel()
    return _NC_CACHE["nc"]


def kernel(x, Wg, bg, W1, b1, W2, b2, W3, b3, _trace=False, _trace_kwargs=None):
    nc = _get_nc()
    in_maps = shard_inputs(x, Wg, bg, W1, b1, W2, b2, W3, b3)
    res = run_bass_kernel_spmd(
        nc, in_maps, list(range(NCORES)), trace=_trace,
        **(_trace_kwargs or {}),
    )
    out = unshard_output(res.results)
    if _trace:
        kernel.last_results = res
    return out



# revision 2
# speedup vs baseline: 1.0237x; 1.0237x over previous
"""Trainium2 Bass kernel for nn_DelayedMLP (B=32, S=2048, I=256, H=512, O=256) — v3.

Structure (per core, 1/8 of the sequence = 256 positions):
  1. Bulk e-phase: e = x * sigmoid(-(x@Wg+bg)) computed ONCE per position
     with fp8 DoubleRow matmuls (K=256/instr).  x8 quantized host-side; Wg
     pre-scaled by 64 into e4m3 range, descaled via the activation `scale`.
  2. Step loop (8 warmup + 16 chunk steps, 16 chains x 32 batch = 512
     tokens/step): u = buf + e; buf-gate matmul fp16; buf = u * sigmoid(-z).
     Chunk steps add c = x + buf_prev - buf and the fused 3-layer fp16 MLP.

Position store layout is r-major: [seg(2), r(16), jj(16), b(32)] + tail
[seg(2), r(8), b(32)] so that bulk tile r is a contiguous 512-token slice
that feeds step t=r directly — the bulk pipeline interleaves with the step
loop instead of serializing ahead of it.  Step-t gathers are contiguous
(t<16) or two contiguous runs (t>=16: 480 from main + 32 from tail).

Emission is software-pipelined: the recurrence ops of step t+1 are emitted
before the MLP ops of step t so the next chain advances on DVE/Scalar while
the Tensor engine runs the current step's MLP.
"""

import numpy as np
import ml_dtypes
from contextlib import ExitStack

import concourse.bass as bass
import concourse.bacc as bacc
import concourse.tile as tile
from concourse import mybir
from concourse.bass_utils import run_bass_kernel_spmd

F32 = mybir.dt.float32
F16 = mybir.dt.float16
F8 = mybir.dt.float8e4
NPF8 = ml_dtypes.float8_e4m3
DR = mybir.MatmulPerfMode.DoubleRow

B, S, I, H, O = 32, 2048, 256, 512, 256
NCORES = 8
CHUNK = S // NCORES          # 256 positions per core
NCHAIN = 16                  # chains per core
CLEN = CHUNK // NCHAIN       # 16 chunk steps per chain
WARM = 6                     # warmup steps per chain
LSTEP = WARM + CLEN          # 24 device steps
TOK = NCHAIN * B             # 512 tokens per device step
FREE = 2 * TOK               # 1024 = two I-segments
MAIN = 16 * 16 * B           # 8192 tokens in the main store block
TAIL = WARM * B              # 256 tokens in the tail block (jj=16, r<WARM)
STORE = MAIN + TAIL          # 8448 tokens per segment
NBULK = 17                   # 16 main r-tiles + 1 tail tile
WSCALE = 64.0                # Wg pre-scale into e4m3 normal range


def build_kernel():
    nc = bacc.Bacc("TRN2", target_bir_lowering=False, debug=False)

    x16_d = nc.dram_tensor("x16", [128, 2 * STORE], F16, kind="ExternalInput").ap()
    x8_d = nc.dram_tensor("x8", [128, 2 * STORE], F8, kind="ExternalInput").ap()
    wg8_d = nc.dram_tensor("Wg8", [I, I], F8, kind="ExternalInput").ap()
    wg_d = nc.dram_tensor("Wg", [I, I], F16, kind="ExternalInput").ap()
    w1_d = nc.dram_tensor("W1", [I, H], F16, kind="ExternalInput").ap()
    w2_d = nc.dram_tensor("W2", [H, H], F16, kind="ExternalInput").ap()
    w3_d = nc.dram_tensor("W3", [H, O], F16, kind="ExternalInput").ap()
    nbg_d = nc.dram_tensor("nbg", [2, 128, 1], F32, kind="ExternalInput").ap()
    b1_d = nc.dram_tensor("b1c", [4, 128, 1], F32, kind="ExternalInput").ap()
    b2_d = nc.dram_tensor("b2c", [4, 128, 1], F32, kind="ExternalInput").ap()
    b3_d = nc.dram_tensor("b3c", [2, 128, 1], F32, kind="ExternalInput").ap()
    outT = nc.dram_tensor("outT", [128, CLEN, FREE], F16, kind="ExternalOutput").ap()

    SIG = mybir.ActivationFunctionType.Sigmoid
    RELU = mybir.ActivationFunctionType.Relu
    IDEN = mybir.ActivationFunctionType.Identity
    ADD = mybir.AluOpType.add
    MAX = mybir.AluOpType.max

    with tile.TileContext(nc) as tc, ExitStack() as ctx:
        wpool = ctx.enter_context(tc.tile_pool(name="weights", bufs=1))
        stpool = ctx.enter_context(tc.tile_pool(name="store", bufs=1))
        sdpool = ctx.enter_context(tc.tile_pool(name="sd", bufs=3))
        upool = ctx.enter_context(tc.tile_pool(name="u", bufs=3))
        spool = ctx.enter_context(tc.tile_pool(name="s", bufs=3))
        bpool = ctx.enter_context(tc.tile_pool(name="buf", bufs=3))
        cpool = ctx.enter_context(tc.tile_pool(name="c", bufs=2))
        ctpool = ctx.enter_context(tc.tile_pool(name="ctmp", bufs=2))
        h1pool = ctx.enter_context(tc.tile_pool(name="h1", bufs=3))
        h2pool = ctx.enter_context(tc.tile_pool(name="h2", bufs=3))
        opool = ctx.enter_context(tc.tile_pool(name="osb", bufs=2))
        pg = ctx.enter_context(tc.tile_pool(name="pg", bufs=4, space="PSUM"))
        pm = ctx.enter_context(tc.tile_pool(name="pm", bufs=4, space="PSUM"))

        # --- resident weights ---
        def load_blocks(src, kk, cols, name, dt):
            t = wpool.tile([128, kk * cols], dt, tag=name, name=name)
            nc.sync.dma_start(
                t[:].rearrange("p (k c) -> p k c", k=kk),
                src.rearrange("(k p) c -> p k c", p=128),
            )
            return t

        wg8t = load_blocks(wg8_d, 2, I, "wg8t", F8)

        def blk(t, cols, k, m):
            return t[:, k * cols + m * 128:k * cols + (m + 1) * 128]

        def blk_dr(t, m):
            return t[:].rearrange("p (k c) -> p k c", k=2)[:, :, m * 128:(m + 1) * 128]

        def load_bias(src, n, name):
            t = wpool.tile([128, n], F32, tag=name, name=name)
            nc.sync.dma_start(
                t[:].rearrange("p (m one) -> p m one", one=1),
                src.rearrange("m p one -> p m one"),
            )
            return [t[:, m:m + 1] for m in range(n)]

        nbg = load_bias(nbg_d, 2, "nbgt")

        # --- position stores ---
        x16 = stpool.tile([128, 2 * STORE], F16, tag="x16", name="x16")
        x8 = stpool.tile([128, 2 * STORE], F8, tag="x8", name="x8")
        est = stpool.tile([128, 2 * STORE], F16, tag="est", name="est")

        def seg3(t, lo, w):
            # [128, 2(seg), w] slice of a store at token offset lo
            return t[:].rearrange("p (s q) -> p s q", s=2)[:, :, lo:lo + w]

        def dma_bulk(i):
            """Chunked input DMA for bulk tile i (so the pipeline streams)."""
            lo, w = (i * 512, 512) if i < 16 else (MAIN, TAIL)
            nc.sync.dma_start(
                seg3(x8, lo, w),
                x8_d.rearrange("p (s q) -> p s q", s=2)[:, :, lo:lo + w],
            )
            nc.sync.dma_start(
                seg3(x16, lo, w),
                x16_d.rearrange("p (s q) -> p s q", s=2)[:, :, lo:lo + w],
            )

        def emit_bulk(i):
            """Bulk tile i: main r-tile (i<16, 512 tokens) or the tail (i==16)."""
            lo, w = (i * 512, 512) if i < 16 else (MAIN, TAIL)
            sd = sdpool.tile([128, 2 * w], F16, tag="sd", name=f"sd{i}")
            for m in range(2):
                zd = pg.tile([128, w], F32, tag="pg", name=f"zd{i}_{m}")
                nc.tensor.matmul(
                    zd[:], blk_dr(wg8t, m), seg3(x8, lo, w),
                    start=True, stop=True, perf_mode=DR,
                )
                nc.scalar.activation(
                    sd[:, m * w:(m + 1) * w], zd[:], SIG,
                    bias=nbg[m], scale=-1.0 / WSCALE,
                )
            nc.vector.tensor_mul(
                seg3(est, lo, w), seg3(x16, lo, w),
                sd[:].rearrange("p (s q) -> p s q", s=2),
            )

        def gathered_binop(op, out_tile, in0_tile, st, t):
            """out[:, seg, jb] = in0 (+/x) store_gather(t), handling the
            t>=16 two-run split (480 main + 32 tail)."""
            def sl(ap, lo, w):
                return ap.rearrange("p (s q) -> p s q", s=2)[:, :, lo:lo + w]
            if t < 16:
                op(sl(out_tile[:], 0, TOK), sl(in0_tile[:], 0, TOK),
                   seg3(st, t * 512, TOK))
            else:
                r = t - 16
                op(sl(out_tile[:], 0, 480), sl(in0_tile[:], 0, 480),
                   seg3(st, r * 512 + 32, 480))
                op(sl(out_tile[:], 480, 32), sl(in0_tile[:], 480, 32),
                   seg3(st, MAIN + r * 32, 32))

        bufs = {}
        half_bufs = {}

        def emit_chain_pair(t):
            """Warm step t for both half-groups (chains 0-7 / 8-15),
            emission interleaved so the two chains pipeline on each engine."""
            us, zzs, ss = [], [], []
            for g in range(2):
                u = upool.tile([128, TOK], F16, tag=f"uh{g}", name=f"u{t}g{g}")
                gsl = seg3(est, t * 512 + g * 256, 256)
                usl = u[:].rearrange("p (s q) -> p s q", s=2)
                if t == 0:
                    nc.vector.tensor_scalar_add(usl, gsl, 0.0)
                else:
                    bsl = half_bufs[(t - 1, g)][:].rearrange("p (s q) -> p s q", s=2)
                    nc.vector.tensor_add(usl, bsl, gsl)
                us.append(u)
            for g in range(2):
                zz = [pm.tile([128, 512], F32, tag="pm", name=f"zz{t}g{g}_{m}")[:, 0:256]
                      for m in range(2)]
                for m in range(2):
                    for k in range(2):
                        nc.tensor.matmul(
                            zz[m], blk(wgt, I, k, m),
                            us[g][:, k * 256:(k + 1) * 256],
                            start=(k == 0), stop=(k == 1),
                        )
                zzs.append(zz)
            for g in range(2):
                s = spool.tile([128, TOK], F16, tag=f"sh{g}", name=f"s{t}g{g}")
                for m in range(2):
                    nc.scalar.activation(
                        s[:, m * 256:(m + 1) * 256], zzs[g][m], SIG,
                        bias=nbg[m], scale=-1.0,
                    )
                ss.append(s)
            for g in range(2):
                buf = bpool.tile([128, TOK], F16, tag=f"bh{g}", name=f"buf{t}g{g}")
                nc.vector.tensor_mul(buf[:], us[g][:], ss[g][:])
                half_bufs[(t, g)] = buf
                half_bufs.pop((t - 2, g), None)

        def emit_chain(t):
            u = upool.tile([128, FREE], F16, tag="u", name=f"u{t}")
            if t == WARM:
                # transition: buf_prev is the pair of warm half-tiles
                for g in range(2):
                    nc.vector.tensor_add(
                        u[:].rearrange("p (s q) -> p s q", s=2)[:, :, g * 256:(g + 1) * 256],
                        half_bufs[(t - 1, g)][:].rearrange("p (s q) -> p s q", s=2),
                        seg3(est, t * 512 + g * 256, 256),
                    )
            else:
                gathered_binop(nc.vector.tensor_add, u, bufs[t - 1], est, t)
            zz = [pg.tile([128, TOK], F32, tag="pg", name=f"zz{t}_{m}") for m in range(2)]
            for m in range(2):
                for k in range(2):
                    nc.tensor.matmul(
                        zz[m][:], blk(wgt, I, k, m),
                        u[:, k * TOK:(k + 1) * TOK],
                        start=(k == 0), stop=(k == 1),
                    )
            s = spool.tile([128, FREE], F16, tag="s", name=f"s{t}")
            for m in range(2):
                nc.scalar.activation(
                    s[:, m * TOK:(m + 1) * TOK], zz[m][:], SIG,
                    bias=nbg[m], scale=-1.0,
                )
            buf = bpool.tile([128, FREE], F16, tag="buf", name=f"buf{t}")
            nc.vector.tensor_mul(buf[:], u[:], s[:])
            bufs[t] = buf
            bufs.pop(t - 3, None)

        cs = {}

        def emit_c(t):
            # ct/c on the recurrence tail so MLP1 isn't gated on the next chain
            ct = ctpool.tile([128, FREE], F16, tag="ctmp", name=f"ct{t}")
            if t == WARM:
                for g in range(2):
                    nc.vector.tensor_sub(
                        ct[:].rearrange("p (s q) -> p s q", s=2)[:, :, g * 256:(g + 1) * 256],
                        half_bufs[(t - 1, g)][:].rearrange("p (s q) -> p s q", s=2),
                        bufs[t][:].rearrange("p (s q) -> p s q", s=2)[:, :, g * 256:(g + 1) * 256],
                    )
            else:
                nc.vector.tensor_sub(ct[:], bufs[t - 1][:], bufs[t][:])
            c = cpool.tile([128, FREE], F16, tag="c", name=f"c{t}")
            gathered_binop(nc.vector.tensor_add, c, ct, x16, t)
            cs[t] = c

        def emit_mlp(t):
            c = cs.pop(t)
            h1 = h1pool.tile([128, 4 * TOK], F16, tag="h1", name=f"h1_{t}")
            for m in range(4):
                ph = pm.tile([128, TOK], F32, tag="pm", name=f"p1_{t}_{m}")
                for k in range(2):
                    nc.tensor.matmul(
                        ph[:], blk(w1t, H, k, m), c[:, k * TOK:(k + 1) * TOK],
                        start=(k == 0), stop=(k == 1),
                    )
                nc.scalar.activation(
                    h1[:, m * TOK:(m + 1) * TOK], ph[:], RELU, bias=b1c[m]
                )

            h2 = h2pool.tile([128, 4 * TOK], F16, tag="h2", name=f"h2_{t}")
            for m in range(4):
                ph = pm.tile([128, TOK], F32, tag="pm", name=f"p2_{t}_{m}")
                for k in range(4):
                    nc.tensor.matmul(
                        ph[:], blk(w2t, H, k, m), h1[:, k * TOK:(k + 1) * TOK],
                        start=(k == 0), stop=(k == 3),
                    )
                if m < 3:
                    nc.vector.tensor_scalar(
                        h2[:, m * TOK:(m + 1) * TOK], ph[:],
                        b2c[m], 0.0, op0=ADD, op1=MAX,
                    )
                else:
                    nc.scalar.activation(
                        h2[:, m * TOK:(m + 1) * TOK], ph[:], RELU, bias=b2c[m]
                    )

            osb = opool.tile([128, FREE], F16, tag="osb", name=f"osb{t}")
            for m in range(2):
                ph = pm.tile([128, TOK], F32, tag="pm", name=f"p3_{t}_{m}")
                for k in range(4):
                    nc.tensor.matmul(
                        ph[:], blk(w3t, O, k, m), h2[:, k * TOK:(k + 1) * TOK],
                        start=(k == 0), stop=(k == 3),
                    )
                if m == 0:
                    nc.vector.tensor_scalar_add(
                        osb[:, m * TOK:(m + 1) * TOK], ph[:], b3c[m]
                    )
                else:
                    nc.scalar.activation(
                        osb[:, m * TOK:(m + 1) * TOK], ph[:], IDEN, bias=b3c[m]
                    )
            nc.sync.dma_start(outT[:, t - WARM, :], osb[:])

        # --- pipelined emission ---
        dma_bulk(0)
        dma_bulk(1)
        wgt = load_blocks(wg_d, 2, I, "wgt", F16)
        emit_bulk(0)
        dma_bulk(2)
        w1t = load_blocks(w1_d, 2, H, "w1t", F16)
        w2t = load_blocks(w2_d, 4, H, "w2t", F16)
        w3t = load_blocks(w3_d, 4, O, "w3t", F16)
        b1c = load_bias(b1_d, 4, "b1t")
        b2c = load_bias(b2_d, 4, "b2t")
        b3c = load_bias(b3_d, 2, "b3t")
        emit_bulk(1)
        emit_chain_pair(0)
        for t in range(LSTEP):
            if t >= WARM:
                emit_c(t)
            if t + 1 < WARM:
                emit_chain_pair(t + 1)
            elif t + 1 < LSTEP:
                emit_chain(t + 1)
            if t + 3 < NBULK:
                dma_bulk(t + 3)
            if t + 2 < NBULK:
                emit_bulk(t + 2)
            if t >= WARM:
                emit_mlp(t)

    nc.compile()
    return nc


def shard_inputs(x, Wg, bg, W1, b1, W2, b2, W3, b3):
    x = np.asarray(x, np.float32)
    xq = np.pad(x, ((0, 0), (WARM, 16 - WARM), (0, 0)))  # [B, WARM + 2048 + (16-WARM), I]

    common = {
        "Wg8": np.ascontiguousarray((np.asarray(Wg, np.float32) * WSCALE).astype(NPF8)),
        "Wg": np.ascontiguousarray(np.asarray(Wg, np.float16)),
        "W1": np.ascontiguousarray(np.asarray(W1, np.float16)),
        "W2": np.ascontiguousarray(np.asarray(W2, np.float16)),
        "W3": np.ascontiguousarray(np.asarray(W3, np.float16)),
        "nbg": np.ascontiguousarray((-np.asarray(bg, np.float32)).reshape(2, 128, 1)),
        "b1c": np.ascontiguousarray(np.asarray(b1, np.float32).reshape(4, 128, 1)),
        "b2c": np.ascontiguousarray(np.asarray(b2, np.float32).reshape(4, 128, 1)),
        "b3c": np.ascontiguousarray(np.asarray(b3, np.float32).reshape(2, 128, 1)),
    }

    in_maps = []
    for k in range(NCORES):
        # store position q (0..271) -> global pos k*256 - 8 + q = xq index k*256 + q
        win = xq[:, k * CHUNK:k * CHUNK + 272, :]          # [B, 272, I]
        win = win.reshape(B, NJJ_ := 17, 16, 2, 128)       # [b, jj, r, seg, p]
        main = win[:, :16].transpose(4, 3, 2, 1, 0)        # [p, seg, r, jj, b]
        tail = win[:, 16, :WARM].transpose(3, 2, 1, 0)     # [p, seg, r(8), b]
        st = np.concatenate(
            [main.reshape(128, 2, MAIN), tail.reshape(128, 2, TAIL)], axis=2
        ).reshape(128, 2 * STORE)
        in_maps.append({
            "x16": np.ascontiguousarray(st.astype(np.float16)),
            "x8": np.ascontiguousarray(st.astype(NPF8)),
            **common,
        })
    return in_maps


def unshard_output(results):
    out = np.empty((B, S, O), np.float32)
    for k in range(NCORES):
        r_ = np.asarray(results[k]["outT"], np.float32).reshape(128, CLEN, 2, NCHAIN, B)
        # [p, tc, oseg, j, b] -> [b, j, tc, oseg, p];  pos = 16*j + tc
        blk = r_.transpose(4, 3, 1, 2, 0).reshape(B, CHUNK, O)
        out[:, k * CHUNK:(k + 1) * CHUNK, :] = blk
    return out


_NC_CACHE = {}


def _get_nc():
    if "nc" not in _NC_CACHE:
        _NC_CACHE["nc"] = build_kernel()
    return _NC_CACHE["nc"]


def kernel(x, Wg, bg, W1, b1, W2, b2, W3, b3, _trace=False, _trace_kwargs=None):
    nc = _get_nc()
    in_maps = shard_inputs(x, Wg, bg, W1, b1, W2, b2, W3, b3)
    res = run_bass_kernel_spmd(
        nc, in_maps, list(range(NCORES)), trace=_trace,
        **(_trace_kwargs or {}),
    )
    out = unshard_output(res.results)
    if _trace:
        kernel.last_results = res
    return out
